# revision 7
# baseline (speedup 1.0000x reference)
"""Mamba block (MockMambaBlock) on 8 Trainium2 NeuronCores.

Sharding: tensor-parallel over d_inner (8 x 256 channels), both batches on
every core. The x_proj/dt_proj contraction over d_inner is completed with an
on-device AllReduce (chunked per 512 tokens so it overlaps phase A); out_proj
row-partials are summed on the host (the gather step).

v2 layout of work across engines:
  - PE: in_proj, depthwise conv (diag-matmul), x_proj, dt_proj, y n-sum
    (identity matmul), D-term (diag matmul), out_proj.
  - Scalar: silu/sigmoid/ln/exp activations, PSUM->SBUF copies.
  - DVE (vector): the 64 SSM scans (f32), dtx, 1/4 of u-multiplies, gating.
  - GPSIMD: 3/4 of the u = dtx*B multiplies (bf16 tensor_tensor).
Batches are pipelined: phase B of batch 0 is emitted interleaved with
phase A of batch 1 so scan work overlaps matmul/activation work.
"""

import sys

sys.path.insert(0, "/opt/trn_rl_repo")

import numpy as np
import ml_dtypes

import concourse.bass as bass
import concourse.bacc as bacc
import concourse.mybir as mybir
import concourse.tile as tile
from concourse.bass_utils import run_bass_kernel_spmd

F32 = mybir.dt.float32
BF16 = mybir.dt.bfloat16
AF = mybir.ActivationFunctionType
OP = mybir.AluOpType

B, L, DM, DI, DS, DC = 2, 2048, 1024, 2048, 16, 4
NCORES = 8
DIL = DI // NCORES          # 256 channels per core
NBLK = DIL // 128           # 2 partition blocks of channels
KBLK = DM // 128            # 8 contraction blocks for in_proj
LTA = 512                   # phase A token chunk
NCHA = L // LTA
NPT = L // 512


def build_nc():
    nc = bacc.Bacc()

    x_t = nc.dram_tensor("x_t", [B, KBLK, 128, L], BF16, kind="ExternalInput")
    win_d = nc.dram_tensor("win", [DM, 2 * DIL], BF16, kind="ExternalInput")
    wout_d = nc.dram_tensor("wout", [DIL, DM], BF16, kind="ExternalInput")
    wx_d = nc.dram_tensor("wx", [DIL, 2 * DS], BF16, kind="ExternalInput")
    wdt_d = nc.dram_tensor("wdt", [DS, DIL], BF16, kind="ExternalInput")
    a_d = nc.dram_tensor("a", [DIL, DS], F32, kind="ExternalInput")
    convdiag_d = nc.dram_tensor("convdiag", [DIL, DC * 128], BF16,
                                kind="ExternalInput")
    convb_d = nc.dram_tensor("convb", [DIL, 1], F32, kind="ExternalInput")
    bdt_d = nc.dram_tensor("bdt", [DIL, 1], F32, kind="ExternalInput")
    identb_d = nc.dram_tensor("identb", [128, 128], BF16, kind="ExternalInput")
    diagd_d = nc.dram_tensor("diagd", [DIL, 128], BF16, kind="ExternalInput")
    out_d = nc.dram_tensor("out_p", [B, L, DM], F32, kind="ExternalOutput")

    with tile.TileContext(nc) as tc:
        with (
            tc.tile_pool(name="weights", bufs=1) as wp,
            tc.tile_pool(name="resident", bufs=1) as rp,
            tc.tile_pool(name="dram", bufs=1, space="DRAM") as dp,
            tc.tile_pool(name="pa", bufs=2) as pa,
            tc.tile_pool(name="pb", bufs=2) as pb,
            tc.tile_pool(name="ps_in", bufs=2, space="PSUM") as ps_in,
            tc.tile_pool(name="ps_cv", bufs=1, space="PSUM") as ps_cv,
            tc.tile_pool(name="ps_small", bufs=1, space="PSUM") as ps_small,
            tc.tile_pool(name="ps_y", bufs=1, space="PSUM") as ps_y,
        ):
            # ---- weights to SBUF ----
            win_sb = wp.tile([128, KBLK, 2 * DIL], BF16)
            nc.sync.dma_start(win_sb[:], win_d[:].rearrange("(k p) m -> p k m", p=128))
            wout_sb = wp.tile([128, NBLK, DM], BF16)
            nc.sync.dma_start(wout_sb[:], wout_d[:].rearrange("(k p) m -> p k m", p=128))
            wx_sb = wp.tile([128, NBLK, 2 * DS], BF16)
            nc.sync.dma_start(wx_sb[:], wx_d[:].rearrange("(k p) m -> p k m", p=128))
            wdt_sb = wp.tile([DS, DIL], BF16)
            nc.sync.dma_start(wdt_sb[:], wdt_d[:])
            a_sb = wp.tile([128, NBLK, DS], F32)
            nc.sync.dma_start(a_sb[:], a_d[:].rearrange("(k p) m -> p k m", p=128))
            convdiag_sb = wp.tile([128, NBLK, DC * 128], BF16)
            nc.sync.dma_start(convdiag_sb[:],
                              convdiag_d[:].rearrange("(k p) m -> p k m", p=128))
            convb_sb = wp.tile([128, NBLK, 1], F32)
            nc.sync.dma_start(convb_sb[:], convb_d[:].rearrange("(k p) m -> p k m", p=128))
            bdt_sb = wp.tile([128, NBLK, 1], F32)
            nc.sync.dma_start(bdt_sb[:], bdt_d[:].rearrange("(k p) m -> p k m", p=128))
            identb_sb = wp.tile([128, 128], BF16)
            nc.sync.dma_start(identb_sb[:], identb_d[:])
            diagd_sb = wp.tile([128, NBLK, 128], BF16)
            nc.sync.dma_start(diagd_sb[:], diagd_d[:].rearrange("(k p) m -> p k m", p=128))

            # ---- resident activations ----
            xcv = [[rp.tile([128, L], BF16, name=f"xcv{b_}{k}", tag=f"xcv{b_}{k}")
                    for k in range(NBLK)] for b_ in range(B)]
            zac = [[rp.tile([128, L], BF16, name=f"zac{b_}{k}", tag=f"zac{b_}{k}")
                    for k in range(NBLK)] for b_ in range(B)]
            dtin_sb = [rp.tile([DS, L], BF16, name=f"dtin{b_}", tag=f"dtin{b_}")
                       for b_ in range(B)]
            md = [[rp.tile([128, L], BF16, name=f"md{b_}{k}", tag=f"md{b_}{k}")
                   for k in range(NBLK)] for b_ in range(B)]

            cc_in = [[dp.tile([2 * DS, LTA], BF16, name=f"cc_in{b_}{ch}")
                      for ch in range(NCHA)] for b_ in range(B)]
            cc_out = [[dp.tile([2 * DS, LTA], BF16, addr_space="Shared",
                               name=f"cc_out{b_}{ch}") for ch in range(NCHA)]
                      for b_ in range(B)]

            xp_buf = [pa.tile([128, LTA + DC - 1], BF16, name=f"xpb{k}",
                              tag=f"xpb{k}", bufs=1) for k in range(NBLK)]

            def emit_phase_a_chunk(b_, ch):
                """in_proj + conv + silu + x_proj + AllReduce + dt for one
                512-token chunk of batch b_."""
                t0 = ch * LTA
                xs_all = pa.tile([128, KBLK, LTA], BF16, tag="xs_all", bufs=3)
                nc.sync.dma_start(
                    xs_all[:], x_t[b_].transpose([1, 0, 2])[:, :, t0:t0 + LTA])
                for m in range(2 * NBLK):
                    ps = ps_in.tile([128, LTA], F32, tag="ps_in")
                    for kb in range(KBLK):
                        nc.tensor.matmul(
                            ps[:], win_sb[:, kb, m * 128:(m + 1) * 128],
                            xs_all[:, kb, :],
                            start=(kb == 0), stop=(kb == KBLK - 1))
                    if m < NBLK:  # x branch: conv + silu
                        blk = m
                        if ch == 0:
                            nc.vector.memset(xp_buf[blk][:, 0:DC - 1], 0.0)
                        else:
                            nc.vector.tensor_copy(
                                xp_buf[blk][:, 0:DC - 1],
                                xp_buf[blk][:, LTA:LTA + DC - 1])
                        nc.scalar.copy(xp_buf[blk][:, DC - 1:LTA + DC - 1], ps[:])
                        psc = ps_cv.tile([128, LTA], F32, tag="ps_cv")
                        for k in range(DC):
                            nc.tensor.matmul(
                                psc[:],
                                convdiag_sb[:, blk, k * 128:(k + 1) * 128],
                                xp_buf[blk][:, k:k + LTA],
                                start=(k == 0), stop=(k == DC - 1))
                        nc.scalar.activation(
                            xcv[b_][blk][:, t0:t0 + LTA], psc[:],
                            AF.Silu, bias=convb_sb[:, blk, :])
                        if m == NBLK - 1:
                            # x_proj partial for this chunk
                            psx = ps_small.tile([128, LTA], F32, tag="ps_small")
                            for kb in range(NBLK):
                                nc.tensor.matmul(
                                    psx[0:2 * DS, :], wx_sb[:, kb, :],
                                    xcv[b_][kb][:, t0:t0 + LTA],
                                    start=(kb == 0), stop=(kb == NBLK - 1))
                            xssb = pa.tile([2 * DS, LTA], BF16, tag="xssb",
                                           bufs=2)
                            nc.scalar.copy(xssb[:], psx[0:2 * DS, :])
                            nc.sync.dma_start(cc_in[b_][ch][:], xssb[:])
                            # chunked AllReduce overlapping the rest of phase A
                            nc.gpsimd.collective_compute(
                                "AllReduce", OP.add,
                                ins=[cc_in[b_][ch].opt()],
                                outs=[cc_out[b_][ch].opt()],
                                replica_groups=[list(range(NCORES))])
                            nc.sync.dma_start(dtin_sb[b_][:, t0:t0 + LTA],
                                              cc_out[b_][ch][0:DS, :])
                            # dt for this chunk:
                            # md = ln(sigmoid(-(dt_raw + b_dt))) = -softplus(.)
                            for blk2 in range(NBLK):
                                psd = ps_small.tile([128, LTA], F32,
                                                    tag="ps_small")
                                nc.tensor.matmul(
                                    psd[:], wdt_sb[:, blk2 * 128:(blk2 + 1) * 128],
                                    dtin_sb[b_][:, t0:t0 + LTA],
                                    start=True, stop=True)
                                nc.scalar.activation(
                                    md[b_][blk2][:, t0:t0 + LTA], psd[:],
                                    AF.Sigmoid, bias=bdt_sb[:, blk2, :],
                                    scale=-1.0)
                                nc.scalar.activation(
                                    md[b_][blk2][:, t0:t0 + LTA],
                                    md[b_][blk2][:, t0:t0 + LTA], AF.Ln)
                    else:  # z branch: silu
                        blk = m - NBLK
                        nc.scalar.activation(
                            zac[b_][blk][:, t0:t0 + LTA], ps[:], AF.Silu)

            def emit_dtx(b_):
                dtx = [pb.tile([128, L], BF16, tag=f"dtx{blk}", bufs=1,
                               name=f"dtx{b_}{blk}") for blk in range(NBLK)]
                for blk in range(NBLK):
                    nc.vector.tensor_mul(dtx[blk][:], md[b_][blk][:],
                                         xcv[b_][blk][:])
                return dtx

            def emit_phase_b_n(b_, blk, n, dtx, y_ps):
                """SSM channel n for one channel-block of batch b_."""
                bb = pb.tile([128, L], BF16, tag="bbn", bufs=2,
                             name=f"bb{b_}{blk}{n}")
                for ch in range(NCHA):
                    nc.sync.dma_start(
                        bb[:, ch * LTA:(ch + 1) * LTA],
                        cc_out[b_][ch][DS + n:DS + n + 1, :]
                        .broadcast_to([128, LTA]))
                # dA_n = exp(A[:, n] * md)   (md = -dt)
                da = pb.tile([128, L], F32, tag="dan", bufs=2,
                             name=f"da{b_}{blk}{n}")
                nc.scalar.activation(da[:], md[b_][blk][:], AF.Exp,
                                     scale=a_sb[:, blk, n:n + 1])
                # u_n = dtx * B_n  (3 of 4 on gpsimd)
                u = pb.tile([128, L], BF16, tag="un", bufs=2,
                            name=f"u{b_}{blk}{n}")
                if n % 4 == 0:
                    nc.vector.tensor_mul(u[:], dtx[blk][:], bb[:])
                else:
                    nc.gpsimd.tensor_mul(u[:], dtx[blk][:], bb[:])
                # full-length scan
                h = pb.tile([128, L], BF16, tag="hn", bufs=3,
                            name=f"h{b_}{blk}{n}")
                nc.vector.tensor_tensor_scan(h[:], da[:], u[:],
                                             0.0, OP.mult, OP.add)
                # y += h_n on the tensor engine (identity matmul)
                for pt in range(NPT):
                    nc.tensor.matmul(
                        y_ps[pt][:], identb_sb[:],
                        h[:, pt * 512:(pt + 1) * 512],
                        start=(n == 0), stop=False)

            def emit_gate(b_, blk, y_ps):
                yin = pb.tile([128, L], BF16, tag=f"yin{blk}", bufs=1,
                              name=f"yin{b_}{blk}")
                for pt in range(NPT):
                    # y += x_conv * D via diag(D) matmul, then gate
                    nc.tensor.matmul(
                        y_ps[pt][:], diagd_sb[:, blk, :],
                        xcv[b_][blk][:, pt * 512:(pt + 1) * 512],
                        start=False, stop=True)
                    nc.vector.tensor_mul(
                        yin[:, pt * 512:(pt + 1) * 512], y_ps[pt][:],
                        zac[b_][blk][:, pt * 512:(pt + 1) * 512])
                return yin

            def emit_out_proj(b_, yins):
                for mt in range(L // 128):
                    for dmh in range(2):
                        ps_o = ps_small.tile([128, 512], F32, tag="ps_small")
                        for blk in range(NBLK):
                            nc.tensor.matmul(
                                ps_o[:],
                                yins[blk][:, mt * 128:(mt + 1) * 128],
                                wout_sb[:, blk, dmh * 512:(dmh + 1) * 512],
                                start=(blk == 0), stop=(blk == NBLK - 1))
                        osb = pb.tile([128, 512], F32, tag="osb")
                        nc.scalar.copy(osb[:], ps_o[:])
                        nc.sync.dma_start(
                            out_d[b_, mt * 128:(mt + 1) * 128,
                                  dmh * 512:(dmh + 1) * 512],
                            osb[:])

            # ================= schedule =================
            for ch in range(NCHA):
                emit_phase_a_chunk(0, ch)

            # interleave: phase B(b0) with phase A(b1) chunks
            dtx0 = emit_dtx(0)
            yins0 = {}
            for blk in range(NBLK):
                y_ps = [ps_y.tile([128, 512], F32, tag=f"y{pt}", bufs=1,
                                  name=f"yps0{blk}{pt}") for pt in range(NPT)]
                for n in range(16):
                    emit_phase_b_n(0, blk, n, dtx0, y_ps)
                    if blk == 0 and n in (1, 5, 9, 13):
                        emit_phase_a_chunk(1, (n - 1) // 4)
                yins0[blk] = emit_gate(0, blk, y_ps)
            emit_out_proj(0, yins0)

            dtx1 = emit_dtx(1)
            yins1 = {}
            for blk in range(NBLK):
                y_ps = [ps_y.tile([128, 512], F32, tag=f"y{pt}", bufs=1,
                                  name=f"yps1{blk}{pt}") for pt in range(NPT)]
                for n in range(16):
                    emit_phase_b_n(1, blk, n, dtx1, y_ps)
                yins1[blk] = emit_gate(1, blk, y_ps)
            emit_out_proj(1, yins1)

    nc.compile()
    return nc


_NC_CACHE = {}


def _get_nc():
    if "nc" not in _NC_CACHE:
        _NC_CACHE["nc"] = build_nc()
    return _NC_CACHE["nc"]


def make_in_maps(x, W_in, conv_w, conv_b, W_x, W_dt, b_dt, A_log, D, W_out):
    x = np.asarray(x, np.float32)
    W_in = np.asarray(W_in, np.float32)
    conv_w = np.asarray(conv_w, np.float32)
    conv_b = np.asarray(conv_b, np.float32)
    W_x = np.asarray(W_x, np.float32)
    W_dt = np.asarray(W_dt, np.float32)
    b_dt = np.asarray(b_dt, np.float32)
    A_log = np.asarray(A_log, np.float32)
    D = np.asarray(D, np.float32)
    W_out = np.asarray(W_out, np.float32)

    xt = np.ascontiguousarray(x.transpose(0, 2, 1)).reshape(B, KBLK, 128, L).astype(ml_dtypes.bfloat16)
    A = np.exp(A_log)  # positive |A|; md = -dt on device

    in_maps = []
    for c in range(NCORES):
        lo = c * DIL
        sl = slice(lo, lo + DIL)
        # diag conv weights: [NBLK, DC, 128, 128] -> [DIL, DC*128]
        cd = np.zeros((NBLK, DC, 128, 128), np.float32)
        for blk in range(NBLK):
            for k in range(DC):
                np.fill_diagonal(cd[blk, k], conv_w[lo + blk * 128:
                                                    lo + (blk + 1) * 128, k])
        cd = cd.transpose(0, 2, 1, 3).reshape(DIL, DC * 128)
        in_maps.append({
            "x_t": xt,
            "win": np.ascontiguousarray(
                np.concatenate([W_in[:, sl], W_in[:, DI + lo:DI + lo + DIL]],
                               axis=1)).astype(ml_dtypes.bfloat16),
            "wout": np.ascontiguousarray(W_out[sl]).astype(ml_dtypes.bfloat16),
            "wx": np.ascontiguousarray(
                np.concatenate([W_x[sl, :DS], -W_x[sl, DS:]], axis=1)
            ).astype(ml_dtypes.bfloat16),
            "wdt": np.ascontiguousarray(W_dt[:, sl]).astype(ml_dtypes.bfloat16),
            "a": np.ascontiguousarray(A[sl]),
            "convdiag": np.ascontiguousarray(cd).astype(ml_dtypes.bfloat16),
            "convb": np.ascontiguousarray(conv_b[sl, None]),
            "bdt": np.ascontiguousarray(-b_dt[sl, None]),
            "identb": np.eye(128, dtype=ml_dtypes.bfloat16),
            "diagd": np.stack([np.diag(D[lo + k * 128:lo + (k + 1) * 128])
                               for k in range(NBLK)]).reshape(DIL, 128)
                       .astype(ml_dtypes.bfloat16),
        })
    return in_maps


def kernel(**inputs):
    nc = _get_nc()
    in_maps = make_in_maps(**inputs)
    res = run_bass_kernel_spmd(nc, in_maps, list(range(NCORES)))
    out = np.zeros((B, L, DM), np.float32)
    for c in range(NCORES):
        out += res.results[c]["out_p"]
    return out


# revision 13
# speedup vs baseline: 1.1373x; 1.1373x over previous
"""Mamba block (MockMambaBlock) on 8 Trainium2 NeuronCores.

Sharding: tensor-parallel over d_inner (8 x 256 channels), both batches on
every core. The x_proj/dt_proj contraction over d_inner is completed with an
on-device AllReduce (chunked per 512 tokens so it overlaps phase A); out_proj
row-partials are summed on the host (the gather step).

v3 layout of work across engines:
  - PE: in_proj, depthwise conv (diag-matmul), x_proj, dt_proj, y n-sum
    (identity matmul), D-term (diag matmul), out_proj.
  - Scalar: silu/softplus/exp activations, PSUM->SBUF copies.
  - DVE (vector): the 64 SSM scans (f32), dtx, 1/4 of u-multiplies, gating.
  - GPSIMD: 3/4 of the u = dtx*B multiplies (bf16 tensor_tensor).
Batches are pipelined: phase B of batch 0 is emitted interleaved with
phase A of batch 1 (compute only; its AllReduces are emitted at the
blk0/blk1 boundary so they never head-of-line-block gpsimd u-muls).
dt uses AF.Softplus with sign-negated A so da = exp(A*dt) directly.
"""

import sys

sys.path.insert(0, "/opt/trn_rl_repo")

import numpy as np
import ml_dtypes

import concourse.bass as bass
import concourse.bacc as bacc
import concourse.mybir as mybir
import concourse.tile as tile
from concourse.bass_utils import run_bass_kernel_spmd

F32 = mybir.dt.float32
BF16 = mybir.dt.bfloat16
AF = mybir.ActivationFunctionType
OP = mybir.AluOpType

B, L, DM, DI, DS, DC = 2, 2048, 1024, 2048, 16, 4
NCORES = 8
DIL = DI // NCORES          # 256 channels per core
NBLK = DIL // 128           # 2 partition blocks of channels
KBLK = DM // 128            # 8 contraction blocks for in_proj
LTA = 512                   # phase A token chunk
NCHA = L // LTA
NPT = L // 512


def build_nc():
    nc = bacc.Bacc()

    x_t = nc.dram_tensor("x_t", [B, KBLK, 128, L], BF16, kind="ExternalInput")
    win_d = nc.dram_tensor("win", [DM, 2 * DIL], BF16, kind="ExternalInput")
    wout_d = nc.dram_tensor("wout", [DIL, DM], BF16, kind="ExternalInput")
    wx_d = nc.dram_tensor("wx", [DIL, 2 * DS], BF16, kind="ExternalInput")
    wdt_d = nc.dram_tensor("wdt", [DS, DIL], BF16, kind="ExternalInput")
    a_d = nc.dram_tensor("a", [DIL, DS], F32, kind="ExternalInput")
    convdiag_d = nc.dram_tensor("convdiag", [DIL, DC * 128], BF16,
                                kind="ExternalInput")
    convb_d = nc.dram_tensor("convb", [DIL, 1], F32, kind="ExternalInput")
    bdt_d = nc.dram_tensor("bdt", [DIL, 1], F32, kind="ExternalInput")
    identb_d = nc.dram_tensor("identb", [128, 128], BF16, kind="ExternalInput")
    diagd_d = nc.dram_tensor("diagd", [DIL, 128], BF16, kind="ExternalInput")
    out_d = nc.dram_tensor("out_p", [B, L, DM], F32, kind="ExternalOutput")

    with tile.TileContext(nc) as tc:
        with (
            tc.tile_pool(name="weights", bufs=1) as wp,
            tc.tile_pool(name="resident", bufs=1) as rp,
            tc.tile_pool(name="dram", bufs=1, space="DRAM") as dp,
            tc.tile_pool(name="pa", bufs=2) as pa,
            tc.tile_pool(name="pb", bufs=2) as pb,
            tc.tile_pool(name="ps_in", bufs=2, space="PSUM") as ps_in,
            tc.tile_pool(name="ps_cv", bufs=1, space="PSUM") as ps_cv,
            tc.tile_pool(name="ps_small", bufs=1, space="PSUM") as ps_small,
            tc.tile_pool(name="ps_y", bufs=1, space="PSUM") as ps_y,
        ):
            # ---- weights to SBUF ----
            win_sb = wp.tile([128, KBLK, 2 * DIL], BF16)
            nc.sync.dma_start(win_sb[:], win_d[:].rearrange("(k p) m -> p k m", p=128))
            wout_sb = wp.tile([128, NBLK, DM], BF16)
            nc.sync.dma_start(wout_sb[:], wout_d[:].rearrange("(k p) m -> p k m", p=128))
            wx_sb = wp.tile([128, NBLK, 2 * DS], BF16)
            nc.sync.dma_start(wx_sb[:], wx_d[:].rearrange("(k p) m -> p k m", p=128))
            wdt_sb = wp.tile([DS, DIL], BF16)
            nc.sync.dma_start(wdt_sb[:], wdt_d[:])
            a_sb = wp.tile([128, NBLK, DS], F32)
            nc.sync.dma_start(a_sb[:], a_d[:].rearrange("(k p) m -> p k m", p=128))
            convdiag_sb = wp.tile([128, NBLK, DC * 128], BF16)
            nc.sync.dma_start(convdiag_sb[:],
                              convdiag_d[:].rearrange("(k p) m -> p k m", p=128))
            convb_sb = wp.tile([128, NBLK, 1], F32)
            nc.sync.dma_start(convb_sb[:], convb_d[:].rearrange("(k p) m -> p k m", p=128))
            bdt_sb = wp.tile([128, NBLK, 1], F32)
            nc.sync.dma_start(bdt_sb[:], bdt_d[:].rearrange("(k p) m -> p k m", p=128))
            identb_sb = wp.tile([128, 128], BF16)
            nc.sync.dma_start(identb_sb[:], identb_d[:])
            diagd_sb = wp.tile([128, NBLK, 128], BF16)
            nc.sync.dma_start(diagd_sb[:], diagd_d[:].rearrange("(k p) m -> p k m", p=128))

            # ---- resident activations ----
            xcv = [[rp.tile([128, L], BF16, name=f"xcv{b_}{k}", tag=f"xcv{b_}{k}")
                    for k in range(NBLK)] for b_ in range(B)]
            zac = [[rp.tile([128, L], BF16, name=f"zac{b_}{k}", tag=f"zac{b_}{k}")
                    for k in range(NBLK)] for b_ in range(B)]
            # AllReduced x_ssm in DRAM: rows 0:DS = dt_in, rows DS: = B_ssm
            # (DRAM so the bb partition-broadcast DMA can read it)
            ccall = [dp.tile([2 * DS, L], BF16, name=f"ccall{b_}")
                     for b_ in range(B)]
            # dt_in rows staged in SBUF for the dt_proj matmul
            dtin_sb = [rp.tile([DS, L], BF16, name=f"dtin{b_}",
                               tag=f"dtin{b_}") for b_ in range(B)]
            md = [[rp.tile([128, L], BF16, name=f"md{b_}{k}", tag=f"md{b_}{k}")
                   for k in range(NBLK)] for b_ in range(B)]

            cc_in = [[dp.tile([2 * DS, LTA], BF16, name=f"cc_in{b_}{ch}")
                      for ch in range(NCHA)] for b_ in range(B)]
            cc_out = [[dp.tile([2 * DS, LTA], BF16, addr_space="Shared",
                               name=f"cc_out{b_}{ch}") for ch in range(NCHA)]
                      for b_ in range(B)]

            xp_buf = [pa.tile([128, LTA + DC - 1], BF16, name=f"xpb{k}",
                              tag=f"xpb{k}", bufs=1) for k in range(NBLK)]

            def emit_a_compute(b_, ch):
                """in_proj + conv + silu + x_proj partial + cc_in DMA for one
                512-token chunk of batch b_. (No collective here.)"""
                t0 = ch * LTA
                xs_all = pa.tile([128, KBLK, LTA], BF16, tag="xs_all", bufs=3)
                nc.sync.dma_start(
                    xs_all[:], x_t[b_].transpose([1, 0, 2])[:, :, t0:t0 + LTA])
                for m in range(2 * NBLK):
                    ps = ps_in.tile([128, LTA], F32, tag="ps_in")
                    for kb in range(KBLK):
                        nc.tensor.matmul(
                            ps[:], win_sb[:, kb, m * 128:(m + 1) * 128],
                            xs_all[:, kb, :],
                            start=(kb == 0), stop=(kb == KBLK - 1))
                    if m < NBLK:  # x branch: conv + silu
                        blk = m
                        if ch == 0:
                            nc.vector.memset(xp_buf[blk][:, 0:DC - 1], 0.0)
                        else:
                            nc.vector.tensor_copy(
                                xp_buf[blk][:, 0:DC - 1],
                                xp_buf[blk][:, LTA:LTA + DC - 1])
                        nc.scalar.copy(xp_buf[blk][:, DC - 1:LTA + DC - 1], ps[:])
                        psc = ps_cv.tile([128, LTA], F32, tag="ps_cv")
                        for k in range(DC):
                            nc.tensor.matmul(
                                psc[:],
                                convdiag_sb[:, blk, k * 128:(k + 1) * 128],
                                xp_buf[blk][:, k:k + LTA],
                                start=(k == 0), stop=(k == DC - 1))
                        nc.scalar.activation(
                            xcv[b_][blk][:, t0:t0 + LTA], psc[:],
                            AF.Silu, bias=convb_sb[:, blk, :])
                        if m == NBLK - 1:
                            # x_proj partial for this chunk
                            psx = ps_small.tile([128, LTA], F32, tag="ps_small")
                            for kb in range(NBLK):
                                nc.tensor.matmul(
                                    psx[0:2 * DS, :], wx_sb[:, kb, :],
                                    xcv[b_][kb][:, t0:t0 + LTA],
                                    start=(kb == 0), stop=(kb == NBLK - 1))
                            xssb = pa.tile([2 * DS, LTA], BF16, tag="xssb",
                                           bufs=2)
                            nc.scalar.copy(xssb[:], psx[0:2 * DS, :])
                            nc.sync.dma_start(cc_in[b_][ch][:], xssb[:])
                    else:  # z branch: silu
                        blk = m - NBLK
                        nc.scalar.activation(
                            zac[b_][blk][:, t0:t0 + LTA], ps[:], AF.Silu)

            def emit_a_reduce(b_, ch):
                """AllReduce for one chunk, repack into ccall, then dt:
                md = softplus(dt_raw + b_dt); A is sign-negated on the host
                so da = exp(A * md) is the decay directly."""
                t0 = ch * LTA
                nc.gpsimd.collective_compute(
                    "AllReduce", OP.add,
                    ins=[cc_in[b_][ch].opt()],
                    outs=[cc_out[b_][ch].opt()],
                    replica_groups=[list(range(NCORES))])
                nc.sync.dma_start(ccall[b_][:, t0:t0 + LTA], cc_out[b_][ch][:])
                nc.sync.dma_start(dtin_sb[b_][:, t0:t0 + LTA],
                                  cc_out[b_][ch][0:DS, :])
                for blk2 in range(NBLK):
                    psd = ps_small.tile([128, LTA], F32, tag="ps_small")
                    nc.tensor.matmul(
                        psd[:], wdt_sb[:, blk2 * 128:(blk2 + 1) * 128],
                        dtin_sb[b_][:, t0:t0 + LTA],
                        start=True, stop=True)
                    # md = ln(sigmoid(-(dt_raw + b_dt))) = -softplus(.)
                    nc.scalar.activation(
                        md[b_][blk2][:, t0:t0 + LTA], psd[:],
                        AF.Sigmoid, bias=bdt_sb[:, blk2, :], scale=-1.0)
                    nc.scalar.activation(
                        md[b_][blk2][:, t0:t0 + LTA],
                        md[b_][blk2][:, t0:t0 + LTA], AF.Ln)

            def emit_dtx(b_):
                dtx = [pb.tile([128, L], BF16, tag=f"dtx{blk}", bufs=1,
                               name=f"dtx{b_}{blk}") for blk in range(NBLK)]
                for blk in range(NBLK):
                    nc.vector.tensor_mul(dtx[blk][:], md[b_][blk][:],
                                         xcv[b_][blk][:])
                return dtx

            def emit_phase_b_n(b_, blk, n, dtx, y_ps):
                """SSM channel n for one channel-block of batch b_."""
                bb = pb.tile([128, L], BF16, tag="bbn", bufs=3,
                             name=f"bb{b_}{blk}{n}")
                nc.sync.dma_start(
                    bb[:], ccall[b_][DS + n:DS + n + 1, :].broadcast_to([128, L]))
                # dA_n = exp(A[:, n] * md)   (md = -dt)
                da = pb.tile([128, L], F32, tag="dan", bufs=2,
                             name=f"da{b_}{blk}{n}")
                nc.scalar.activation(da[:], md[b_][blk][:], AF.Exp,
                                     scale=a_sb[:, blk, n:n + 1])
                # u_n = dtx * B_n  (3 of 4 on gpsimd)
                u = pb.tile([128, L], BF16, tag="un", bufs=3,
                            name=f"u{b_}{blk}{n}")
                if n % 4 == 0:
                    nc.vector.tensor_mul(u[:], dtx[blk][:], bb[:])
                else:
                    nc.gpsimd.tensor_mul(u[:], dtx[blk][:], bb[:])
                # full-length scan
                h = pb.tile([128, L], BF16, tag="hn", bufs=3,
                            name=f"h{b_}{blk}{n}")
                nc.vector.tensor_tensor_scan(h[:], da[:], u[:],
                                             0.0, OP.mult, OP.add)
                # y += h_n on the tensor engine (identity matmul)
                for pt in range(NPT):
                    nc.tensor.matmul(
                        y_ps[pt][:], identb_sb[:],
                        h[:, pt * 512:(pt + 1) * 512],
                        start=(n == 0), stop=False)

            def emit_gate(b_, blk, y_ps):
                yin = pb.tile([128, L], BF16, tag=f"yin{blk}", bufs=1,
                              name=f"yin{b_}{blk}")
                for pt in range(NPT):
                    # y += x_conv * D via diag(D) matmul, then gate
                    nc.tensor.matmul(
                        y_ps[pt][:], diagd_sb[:, blk, :],
                        xcv[b_][blk][:, pt * 512:(pt + 1) * 512],
                        start=False, stop=True)
                    nc.vector.tensor_mul(
                        yin[:, pt * 512:(pt + 1) * 512], y_ps[pt][:],
                        zac[b_][blk][:, pt * 512:(pt + 1) * 512])
                return yin

            def emit_out_proj(b_, yins):
                for mt in range(L // 128):
                    for dmh in range(2):
                        ps_o = ps_small.tile([128, 512], F32, tag="ps_small")
                        for blk in range(NBLK):
                            nc.tensor.matmul(
                                ps_o[:],
                                yins[blk][:, mt * 128:(mt + 1) * 128],
                                wout_sb[:, blk, dmh * 512:(dmh + 1) * 512],
                                start=(blk == 0), stop=(blk == NBLK - 1))
                        osb = pb.tile([128, 512], F32, tag="osb")
                        nc.scalar.copy(osb[:], ps_o[:])
                        nc.sync.dma_start(
                            out_d[b_, mt * 128:(mt + 1) * 128,
                                  dmh * 512:(dmh + 1) * 512],
                            osb[:])

            # ================= schedule =================
            for ch in range(NCHA):
                emit_a_compute(0, ch)
                emit_a_reduce(0, ch)

            # interleave: phase B(b0) with phase A(b1) compute chunks
            dtx0 = emit_dtx(0)
            yins0 = {}
            for blk in range(NBLK):
                y_ps = [ps_y.tile([128, 512], F32, tag=f"y{pt}", bufs=1,
                                  name=f"yps0{blk}{pt}") for pt in range(NPT)]
                for n in range(16):
                    emit_phase_b_n(0, blk, n, dtx0, y_ps)
                    if blk == 0 and n in (1, 5, 9, 13):
                        emit_a_compute(1, (n - 1) // 4)
                yins0[blk] = emit_gate(0, blk, y_ps)
                if blk == 0:
                    # A(b1) collectives: inputs are ready by now, so they
                    # retire instantly and never block gpsimd u-muls.
                    for ch in range(NCHA):
                        emit_a_reduce(1, ch)
            emit_out_proj(0, yins0)

            dtx1 = emit_dtx(1)
            yins1 = {}
            for blk in range(NBLK):
                y_ps = [ps_y.tile([128, 512], F32, tag=f"y{pt}", bufs=1,
                                  name=f"yps1{blk}{pt}") for pt in range(NPT)]
                for n in range(16):
                    emit_phase_b_n(1, blk, n, dtx1, y_ps)
                yins1[blk] = emit_gate(1, blk, y_ps)
            emit_out_proj(1, yins1)

    nc.compile()
    return nc


_NC_CACHE = {}


def _get_nc():
    if "nc" not in _NC_CACHE:
        _NC_CACHE["nc"] = build_nc()
    return _NC_CACHE["nc"]


def make_in_maps(x, W_in, conv_w, conv_b, W_x, W_dt, b_dt, A_log, D, W_out):
    x = np.asarray(x, np.float32)
    W_in = np.asarray(W_in, np.float32)
    conv_w = np.asarray(conv_w, np.float32)
    conv_b = np.asarray(conv_b, np.float32)
    W_x = np.asarray(W_x, np.float32)
    W_dt = np.asarray(W_dt, np.float32)
    b_dt = np.asarray(b_dt, np.float32)
    A_log = np.asarray(A_log, np.float32)
    D = np.asarray(D, np.float32)
    W_out = np.asarray(W_out, np.float32)

    xt = np.ascontiguousarray(x.transpose(0, 2, 1)).reshape(B, KBLK, 128, L).astype(ml_dtypes.bfloat16)
    A = np.exp(A_log)  # positive |A|; md = -softplus(dt) on device

    in_maps = []
    for c in range(NCORES):
        lo = c * DIL
        sl = slice(lo, lo + DIL)
        # diag conv weights: [NBLK, DC, 128, 128] -> [DIL, DC*128]
        cd = np.zeros((NBLK, DC, 128, 128), np.float32)
        for blk in range(NBLK):
            for k in range(DC):
                np.fill_diagonal(cd[blk, k], conv_w[lo + blk * 128:
                                                    lo + (blk + 1) * 128, k])
        cd = cd.transpose(0, 2, 1, 3).reshape(DIL, DC * 128)
        in_maps.append({
            "x_t": xt,
            "win": np.ascontiguousarray(
                np.concatenate([W_in[:, sl], W_in[:, DI + lo:DI + lo + DIL]],
                               axis=1)).astype(ml_dtypes.bfloat16),
            "wout": np.ascontiguousarray(W_out[sl]).astype(ml_dtypes.bfloat16),
            "wx": np.ascontiguousarray(
                np.concatenate([W_x[sl, :DS], -W_x[sl, DS:]], axis=1)
            ).astype(ml_dtypes.bfloat16),
            "wdt": np.ascontiguousarray(W_dt[:, sl]).astype(ml_dtypes.bfloat16),
            "a": np.ascontiguousarray(A[sl]),
            "convdiag": np.ascontiguousarray(cd).astype(ml_dtypes.bfloat16),
            "convb": np.ascontiguousarray(conv_b[sl, None]),
            "bdt": np.ascontiguousarray(-b_dt[sl, None]),
            "identb": np.eye(128, dtype=ml_dtypes.bfloat16),
            "diagd": np.stack([np.diag(D[lo + k * 128:lo + (k + 1) * 128])
                               for k in range(NBLK)]).reshape(DIL, 128)
                       .astype(ml_dtypes.bfloat16),
        })
    return in_maps


def kernel(**inputs):
    nc = _get_nc()
    in_maps = make_in_maps(**inputs)
    res = run_bass_kernel_spmd(nc, in_maps, list(range(NCORES)))
    out = np.zeros((B, L, DM), np.float32)
    for c in range(NCORES):
        out += res.results[c]["out_p"]
    return out


# revision 15
# speedup vs baseline: 1.1613x; 1.0211x over previous
"""Mamba block (MockMambaBlock) on 8 Trainium2 NeuronCores.

Sharding: tensor-parallel over d_inner (8 x 256 channels), both batches on
every core. The x_proj/dt_proj contraction over d_inner is completed with an
on-device AllReduce (chunked per 512 tokens so it overlaps phase A); out_proj
row-partials are summed on the host (the gather step).

v3 layout of work across engines:
  - PE: in_proj, depthwise conv (diag-matmul), x_proj, dt_proj, y n-sum
    (identity matmul), D-term (diag matmul), out_proj.
  - Scalar: silu/softplus/exp activations, PSUM->SBUF copies.
  - DVE (vector): the 64 SSM scans (f32), dtx, 1/4 of u-multiplies, gating.
  - GPSIMD: 3/4 of the u = dtx*B multiplies (bf16 tensor_tensor).
Batches are pipelined: phase B of batch 0 is emitted interleaved with
phase A of batch 1 (compute only; its AllReduces are emitted at the
blk0/blk1 boundary so they never head-of-line-block gpsimd u-muls).
dt uses AF.Softplus with sign-negated A so da = exp(A*dt) directly.
"""

import sys

sys.path.insert(0, "/opt/trn_rl_repo")

import numpy as np
import ml_dtypes

import concourse.bass as bass
import concourse.bacc as bacc
import concourse.mybir as mybir
import concourse.tile as tile
from concourse.bass_utils import run_bass_kernel_spmd

F32 = mybir.dt.float32
BF16 = mybir.dt.bfloat16
AF = mybir.ActivationFunctionType
OP = mybir.AluOpType

B, L, DM, DI, DS, DC = 2, 2048, 1024, 2048, 16, 4
NCORES = 8
DIL = DI // NCORES          # 256 channels per core
NBLK = DIL // 128           # 2 partition blocks of channels
KBLK = DM // 128            # 8 contraction blocks for in_proj
LTA = 512                   # phase A token chunk
NCHA = L // LTA
NPT = L // 512


def build_nc():
    nc = bacc.Bacc()

    x_t = nc.dram_tensor("x_t", [B, KBLK, 128, L], BF16, kind="ExternalInput")
    win_d = nc.dram_tensor("win", [DM, 2 * DIL], BF16, kind="ExternalInput")
    wout_d = nc.dram_tensor("wout", [DIL, DM], BF16, kind="ExternalInput")
    wx_d = nc.dram_tensor("wx", [DIL, 2 * DS], BF16, kind="ExternalInput")
    wdt_d = nc.dram_tensor("wdt", [DS, DIL], BF16, kind="ExternalInput")
    a_d = nc.dram_tensor("a", [DIL, DS], F32, kind="ExternalInput")
    convdiag_d = nc.dram_tensor("convdiag", [DIL, DC * 128], BF16,
                                kind="ExternalInput")
    convb_d = nc.dram_tensor("convb", [DIL, 1], F32, kind="ExternalInput")
    bdt_d = nc.dram_tensor("bdt", [DIL, 1], F32, kind="ExternalInput")
    identb_d = nc.dram_tensor("identb", [128, 128], BF16, kind="ExternalInput")
    diagd_d = nc.dram_tensor("diagd", [DIL, 128], BF16, kind="ExternalInput")
    out_d = nc.dram_tensor("out_p", [B, L, DM], F32, kind="ExternalOutput")

    with tile.TileContext(nc) as tc:
        with (
            tc.tile_pool(name="weights", bufs=1) as wp,
            tc.tile_pool(name="resident", bufs=1) as rp,
            tc.tile_pool(name="dram", bufs=1, space="DRAM") as dp,
            tc.tile_pool(name="pa", bufs=2) as pa,
            tc.tile_pool(name="pb", bufs=2) as pb,
            tc.tile_pool(name="ps_in", bufs=2, space="PSUM") as ps_in,
            tc.tile_pool(name="ps_cv", bufs=1, space="PSUM") as ps_cv,
            tc.tile_pool(name="ps_small", bufs=1, space="PSUM") as ps_small,
            tc.tile_pool(name="ps_y", bufs=1, space="PSUM") as ps_y,
        ):
            # ---- weights to SBUF ----
            win_sb = wp.tile([128, KBLK, 2 * DIL], BF16)
            nc.sync.dma_start(win_sb[:], win_d[:].rearrange("(k p) m -> p k m", p=128))
            wout_sb = wp.tile([128, NBLK, DM], BF16)
            nc.sync.dma_start(wout_sb[:], wout_d[:].rearrange("(k p) m -> p k m", p=128))
            wx_sb = wp.tile([128, NBLK, 2 * DS], BF16)
            nc.sync.dma_start(wx_sb[:], wx_d[:].rearrange("(k p) m -> p k m", p=128))
            wdt_sb = wp.tile([DS, DIL], BF16)
            nc.sync.dma_start(wdt_sb[:], wdt_d[:])
            a_sb = wp.tile([128, NBLK, DS], F32)
            nc.sync.dma_start(a_sb[:], a_d[:].rearrange("(k p) m -> p k m", p=128))
            convdiag_sb = wp.tile([128, NBLK, DC * 128], BF16)
            nc.sync.dma_start(convdiag_sb[:],
                              convdiag_d[:].rearrange("(k p) m -> p k m", p=128))
            convb_sb = wp.tile([128, NBLK, 1], F32)
            nc.sync.dma_start(convb_sb[:], convb_d[:].rearrange("(k p) m -> p k m", p=128))
            bdt_sb = wp.tile([128, NBLK, 1], F32)
            nc.sync.dma_start(bdt_sb[:], bdt_d[:].rearrange("(k p) m -> p k m", p=128))
            identb_sb = wp.tile([128, 128], BF16)
            nc.sync.dma_start(identb_sb[:], identb_d[:])
            diagd_sb = wp.tile([128, NBLK, 128], BF16)
            nc.sync.dma_start(diagd_sb[:], diagd_d[:].rearrange("(k p) m -> p k m", p=128))

            # ---- resident activations ----
            xcv = [[rp.tile([128, L], BF16, name=f"xcv{b_}{k}", tag=f"xcv{b_}{k}")
                    for k in range(NBLK)] for b_ in range(B)]
            zac = [[rp.tile([128, L], BF16, name=f"zac{b_}{k}", tag=f"zac{b_}{k}")
                    for k in range(NBLK)] for b_ in range(B)]
            # AllReduced x_ssm in DRAM: rows 0:DS = dt_in, rows DS: = B_ssm
            # (DRAM so the bb partition-broadcast DMA can read it)
            ccall = [dp.tile([2 * DS, L], BF16, name=f"ccall{b_}")
                     for b_ in range(B)]
            # dt_in rows staged in SBUF for the dt_proj matmul
            dtin_sb = [rp.tile([DS, L], BF16, name=f"dtin{b_}",
                               tag=f"dtin{b_}") for b_ in range(B)]
            md = [[rp.tile([128, L], BF16, name=f"md{b_}{k}", tag=f"md{b_}{k}")
                   for k in range(NBLK)] for b_ in range(B)]

            cc_in = [[dp.tile([2 * DS, LTA], BF16, name=f"cc_in{b_}{ch}")
                      for ch in range(NCHA)] for b_ in range(B)]
            cc_out = [[dp.tile([2 * DS, LTA], BF16, addr_space="Shared",
                               name=f"cc_out{b_}{ch}") for ch in range(NCHA)]
                      for b_ in range(B)]

            xp_buf = [pa.tile([128, LTA + DC - 1], BF16, name=f"xpb{k}",
                              tag=f"xpb{k}", bufs=1) for k in range(NBLK)]

            def emit_a_compute(b_, ch):
                """in_proj + conv + silu + x_proj partial + cc_in DMA for one
                512-token chunk of batch b_. (No collective here.)"""
                t0 = ch * LTA
                xs_all = pa.tile([128, KBLK, LTA], BF16, tag="xs_all", bufs=3)
                nc.sync.dma_start(
                    xs_all[:], x_t[b_].transpose([1, 0, 2])[:, :, t0:t0 + LTA])
                for m in range(2 * NBLK):
                    ps = ps_in.tile([128, LTA], F32, tag="ps_in")
                    for kb in range(KBLK):
                        nc.tensor.matmul(
                            ps[:], win_sb[:, kb, m * 128:(m + 1) * 128],
                            xs_all[:, kb, :],
                            start=(kb == 0), stop=(kb == KBLK - 1))
                    if m < NBLK:  # x branch: conv + silu
                        blk = m
                        if ch == 0:
                            nc.vector.memset(xp_buf[blk][:, 0:DC - 1], 0.0)
                        else:
                            nc.vector.tensor_copy(
                                xp_buf[blk][:, 0:DC - 1],
                                xp_buf[blk][:, LTA:LTA + DC - 1])
                        nc.scalar.copy(xp_buf[blk][:, DC - 1:LTA + DC - 1], ps[:])
                        psc = ps_cv.tile([128, LTA], F32, tag="ps_cv")
                        for k in range(DC):
                            nc.tensor.matmul(
                                psc[:],
                                convdiag_sb[:, blk, k * 128:(k + 1) * 128],
                                xp_buf[blk][:, k:k + LTA],
                                start=(k == 0), stop=(k == DC - 1))
                        nc.scalar.activation(
                            xcv[b_][blk][:, t0:t0 + LTA], psc[:],
                            AF.Silu, bias=convb_sb[:, blk, :])
                        if m == NBLK - 1:
                            # x_proj partial for this chunk
                            psx = ps_small.tile([128, LTA], F32, tag="ps_small")
                            for kb in range(NBLK):
                                nc.tensor.matmul(
                                    psx[0:2 * DS, :], wx_sb[:, kb, :],
                                    xcv[b_][kb][:, t0:t0 + LTA],
                                    start=(kb == 0), stop=(kb == NBLK - 1))
                            xssb = pa.tile([2 * DS, LTA], BF16, tag="xssb",
                                           bufs=2)
                            nc.scalar.copy(xssb[:], psx[0:2 * DS, :])
                            nc.sync.dma_start(cc_in[b_][ch][:], xssb[:])
                    else:  # z branch: silu
                        blk = m - NBLK
                        nc.scalar.activation(
                            zac[b_][blk][:, t0:t0 + LTA], ps[:], AF.Silu)

            def emit_a_reduce(b_, ch):
                """AllReduce for one chunk, repack into ccall, then dt:
                md = softplus(dt_raw + b_dt); A is sign-negated on the host
                so da = exp(A * md) is the decay directly."""
                t0 = ch * LTA
                nc.gpsimd.collective_compute(
                    "AllReduce", OP.add,
                    ins=[cc_in[b_][ch].opt()],
                    outs=[cc_out[b_][ch].opt()],
                    replica_groups=[list(range(NCORES))])
                nc.sync.dma_start(ccall[b_][:, t0:t0 + LTA], cc_out[b_][ch][:])
                nc.sync.dma_start(dtin_sb[b_][:, t0:t0 + LTA],
                                  cc_out[b_][ch][0:DS, :])
                for blk2 in range(NBLK):
                    psd = ps_small.tile([128, LTA], F32, tag="ps_small")
                    nc.tensor.matmul(
                        psd[:], wdt_sb[:, blk2 * 128:(blk2 + 1) * 128],
                        dtin_sb[b_][:, t0:t0 + LTA],
                        start=True, stop=True)
                    # md = ln(sigmoid(-(dt_raw + b_dt))) = -softplus(.)
                    nc.scalar.activation(
                        md[b_][blk2][:, t0:t0 + LTA], psd[:],
                        AF.Sigmoid, bias=bdt_sb[:, blk2, :], scale=-1.0)
                    nc.scalar.activation(
                        md[b_][blk2][:, t0:t0 + LTA],
                        md[b_][blk2][:, t0:t0 + LTA], AF.Ln)

            HL = L // 2

            def emit_dtx(b_):
                """dtx = md * xcv, emitted per L-half so the low half is
                available as soon as md's first two chunks land."""
                dtx = [pb.tile([128, L], BF16, tag=f"dtx{blk}", bufs=1,
                               name=f"dtx{b_}{blk}") for blk in range(NBLK)]
                for half in range(2):
                    s = slice(half * HL, (half + 1) * HL)
                    for blk in range(NBLK):
                        nc.vector.tensor_mul(dtx[blk][:, s], md[b_][blk][:, s],
                                             xcv[b_][blk][:, s])
                return dtx

            def emit_phase_b_n(b_, blk, n, dtx, y_ps):
                """SSM channel n for one channel-block of batch b_,
                processed in chained L-halves for earlier pipeline start."""
                bb = pb.tile([128, L], BF16, tag="bbn", bufs=3,
                             name=f"bb{b_}{blk}{n}")
                da = pb.tile([128, L], F32, tag="dan", bufs=2,
                             name=f"da{b_}{blk}{n}")
                u = pb.tile([128, L], BF16, tag="un", bufs=3,
                            name=f"u{b_}{blk}{n}")
                h = pb.tile([128, L], BF16, tag="hn", bufs=3,
                            name=f"h{b_}{blk}{n}")
                for half in range(2):
                    s = slice(half * HL, (half + 1) * HL)
                    nc.sync.dma_start(
                        bb[:, s],
                        ccall[b_][DS + n:DS + n + 1, s].broadcast_to([128, HL]))
                    # dA_n = exp(A[:, n] * md)   (md = -dt)
                    nc.scalar.activation(da[:, s], md[b_][blk][:, s], AF.Exp,
                                         scale=a_sb[:, blk, n:n + 1])
                    # u_n = dtx * B_n  (7 of 8 on gpsimd)
                    if n % 8 == 4:
                        nc.vector.tensor_mul(u[:, s], dtx[blk][:, s], bb[:, s])
                    else:
                        nc.gpsimd.tensor_mul(u[:, s], dtx[blk][:, s], bb[:, s])
                    # chained scan halves
                    nc.vector.tensor_tensor_scan(
                        h[:, s], da[:, s], u[:, s],
                        0.0 if half == 0 else h[:, HL - 1:HL],
                        OP.mult, OP.add)
                    # y += h_n on the tensor engine (identity matmul)
                    for pt in (0, 1):
                        gpt = half * 2 + pt
                        nc.tensor.matmul(
                            y_ps[gpt][:], identb_sb[:],
                            h[:, gpt * 512:(gpt + 1) * 512],
                            start=(n == 0), stop=False)

            def emit_gate(b_, blk, y_ps):
                yin = pb.tile([128, L], BF16, tag=f"yin{blk}", bufs=1,
                              name=f"yin{b_}{blk}")
                for pt in range(NPT):
                    # y += x_conv * D via diag(D) matmul, then gate
                    nc.tensor.matmul(
                        y_ps[pt][:], diagd_sb[:, blk, :],
                        xcv[b_][blk][:, pt * 512:(pt + 1) * 512],
                        start=False, stop=True)
                    nc.vector.tensor_mul(
                        yin[:, pt * 512:(pt + 1) * 512], y_ps[pt][:],
                        zac[b_][blk][:, pt * 512:(pt + 1) * 512])
                return yin

            def emit_out_proj(b_, yins):
                # pipelined through the (now free) y PSUM banks, 2 per mt
                for mt in range(L // 128):
                    pso = [ps_y.tile([128, 512], F32, tag=f"y{(2 * mt + i) % 4}",
                                     name=f"pso{b_}{mt}{i}") for i in range(2)]
                    for dmh in range(2):
                        for blk in range(NBLK):
                            nc.tensor.matmul(
                                pso[dmh][:],
                                yins[blk][:, mt * 128:(mt + 1) * 128],
                                wout_sb[:, blk, dmh * 512:(dmh + 1) * 512],
                                start=(blk == 0), stop=(blk == NBLK - 1))
                    osb = pb.tile([128, DM], F32, tag="osb")
                    nc.scalar.copy(osb[:, 0:512], pso[0][:])
                    nc.scalar.copy(osb[:, 512:DM], pso[1][:])
                    nc.sync.dma_start(
                        out_d[b_, mt * 128:(mt + 1) * 128, :], osb[:])

            # ================= schedule =================
            for ch in range(NCHA):
                emit_a_compute(0, ch)
                emit_a_reduce(0, ch)

            # interleave: phase B(b0) with phase A(b1) compute chunks
            dtx0 = emit_dtx(0)
            yins0 = {}
            for blk in range(NBLK):
                y_ps = [ps_y.tile([128, 512], F32, tag=f"y{pt}", bufs=1,
                                  name=f"yps0{blk}{pt}") for pt in range(NPT)]
                for n in range(16):
                    emit_phase_b_n(0, blk, n, dtx0, y_ps)
                    if blk == 0 and n in (1, 5, 9, 13):
                        emit_a_compute(1, (n - 1) // 4)
                yins0[blk] = emit_gate(0, blk, y_ps)
                if blk == 0:
                    # A(b1) collectives: inputs are ready by now, so they
                    # retire instantly and never block gpsimd u-muls.
                    for ch in range(NCHA):
                        emit_a_reduce(1, ch)
            emit_out_proj(0, yins0)

            dtx1 = emit_dtx(1)
            yins1 = {}
            for blk in range(NBLK):
                y_ps = [ps_y.tile([128, 512], F32, tag=f"y{pt}", bufs=1,
                                  name=f"yps1{blk}{pt}") for pt in range(NPT)]
                for n in range(16):
                    emit_phase_b_n(1, blk, n, dtx1, y_ps)
                yins1[blk] = emit_gate(1, blk, y_ps)
            emit_out_proj(1, yins1)

    nc.compile()
    return nc


_NC_CACHE = {}


def _get_nc():
    if "nc" not in _NC_CACHE:
        _NC_CACHE["nc"] = build_nc()
    return _NC_CACHE["nc"]


def make_in_maps(x, W_in, conv_w, conv_b, W_x, W_dt, b_dt, A_log, D, W_out):
    x = np.asarray(x, np.float32)
    W_in = np.asarray(W_in, np.float32)
    conv_w = np.asarray(conv_w, np.float32)
    conv_b = np.asarray(conv_b, np.float32)
    W_x = np.asarray(W_x, np.float32)
    W_dt = np.asarray(W_dt, np.float32)
    b_dt = np.asarray(b_dt, np.float32)
    A_log = np.asarray(A_log, np.float32)
    D = np.asarray(D, np.float32)
    W_out = np.asarray(W_out, np.float32)

    xt = np.ascontiguousarray(x.transpose(0, 2, 1)).reshape(B, KBLK, 128, L).astype(ml_dtypes.bfloat16)
    A = np.exp(A_log)  # positive |A|; md = -softplus(dt) on device

    in_maps = []
    for c in range(NCORES):
        lo = c * DIL
        sl = slice(lo, lo + DIL)
        # diag conv weights: [NBLK, DC, 128, 128] -> [DIL, DC*128]
        cd = np.zeros((NBLK, DC, 128, 128), np.float32)
        for blk in range(NBLK):
            for k in range(DC):
                np.fill_diagonal(cd[blk, k], conv_w[lo + blk * 128:
                                                    lo + (blk + 1) * 128, k])
        cd = cd.transpose(0, 2, 1, 3).reshape(DIL, DC * 128)
        in_maps.append({
            "x_t": xt,
            "win": np.ascontiguousarray(
                np.concatenate([W_in[:, sl], W_in[:, DI + lo:DI + lo + DIL]],
                               axis=1)).astype(ml_dtypes.bfloat16),
            "wout": np.ascontiguousarray(W_out[sl]).astype(ml_dtypes.bfloat16),
            "wx": np.ascontiguousarray(
                np.concatenate([W_x[sl, :DS], -W_x[sl, DS:]], axis=1)
            ).astype(ml_dtypes.bfloat16),
            "wdt": np.ascontiguousarray(W_dt[:, sl]).astype(ml_dtypes.bfloat16),
            "a": np.ascontiguousarray(A[sl]),
            "convdiag": np.ascontiguousarray(cd).astype(ml_dtypes.bfloat16),
            "convb": np.ascontiguousarray(conv_b[sl, None]),
            "bdt": np.ascontiguousarray(-b_dt[sl, None]),
            "identb": np.eye(128, dtype=ml_dtypes.bfloat16),
            "diagd": np.stack([np.diag(D[lo + k * 128:lo + (k + 1) * 128])
                               for k in range(NBLK)]).reshape(DIL, 128)
                       .astype(ml_dtypes.bfloat16),
        })
    return in_maps


def kernel(**inputs):
    nc = _get_nc()
    in_maps = make_in_maps(**inputs)
    res = run_bass_kernel_spmd(nc, in_maps, list(range(NCORES)))
    out = np.zeros((B, L, DM), np.float32)
    for c in range(NCORES):
        out += res.results[c]["out_p"]
    return out


# revision 16
# speedup vs baseline: 1.3580x; 1.1693x over previous
"""Mamba block (MockMambaBlock) on 8 Trainium2 NeuronCores.

Sharding: tensor-parallel over d_inner (8 x 256 channels), both batches on
every core. The x_proj/dt_proj contraction over d_inner is completed with an
on-device AllReduce (chunked per 512 tokens so it overlaps phase A); out_proj
row-partials are summed on the host (the gather step).

v3 layout of work across engines:
  - PE: in_proj, depthwise conv (diag-matmul), x_proj, dt_proj, y n-sum
    (identity matmul), D-term (diag matmul), out_proj.
  - Scalar: silu/softplus/exp activations, PSUM->SBUF copies.
  - DVE (vector): the 64 SSM scans (f32), dtx, 1/4 of u-multiplies, gating.
  - GPSIMD: 3/4 of the u = dtx*B multiplies (bf16 tensor_tensor).
Batches are pipelined: phase B of batch 0 is emitted interleaved with
phase A of batch 1 (compute only; its AllReduces are emitted at the
blk0/blk1 boundary so they never head-of-line-block gpsimd u-muls).
dt uses AF.Softplus with sign-negated A so da = exp(A*dt) directly.
"""

import sys

sys.path.insert(0, "/opt/trn_rl_repo")

import numpy as np
import ml_dtypes

import concourse.bass as bass
import concourse.bacc as bacc
import concourse.mybir as mybir
import concourse.tile as tile
from concourse.bass_utils import run_bass_kernel_spmd

F32 = mybir.dt.float32
BF16 = mybir.dt.bfloat16
AF = mybir.ActivationFunctionType
OP = mybir.AluOpType

B, L, DM, DI, DS, DC = 2, 2048, 1024, 2048, 16, 4
NCORES = 8
DIL = DI // NCORES          # 256 channels per core
NBLK = DIL // 128           # 2 partition blocks of channels
KBLK = DM // 128            # 8 contraction blocks for in_proj
LTA = 512                   # phase A token chunk
NCHA = L // LTA
NPT = L // 512


def build_nc():
    nc = bacc.Bacc()

    x_t = nc.dram_tensor("x_t", [B, KBLK, 128, L], BF16, kind="ExternalInput")
    win_d = nc.dram_tensor("win", [DM, 2 * DIL], BF16, kind="ExternalInput")
    wout_d = nc.dram_tensor("wout", [DIL, DM], BF16, kind="ExternalInput")
    wx_d = nc.dram_tensor("wx", [DIL, 2 * DS], BF16, kind="ExternalInput")
    wdt_d = nc.dram_tensor("wdt", [DS, DIL], BF16, kind="ExternalInput")
    a_d = nc.dram_tensor("a", [DIL, DS], F32, kind="ExternalInput")
    convdiag_d = nc.dram_tensor("convdiag", [DIL, DC * 128], BF16,
                                kind="ExternalInput")
    convb_d = nc.dram_tensor("convb", [DIL, 1], F32, kind="ExternalInput")
    bdt_d = nc.dram_tensor("bdt", [DIL, 1], F32, kind="ExternalInput")
    identb_d = nc.dram_tensor("identb", [128, 128], BF16, kind="ExternalInput")
    diagd_d = nc.dram_tensor("diagd", [DIL, 128], BF16, kind="ExternalInput")
    out_d = nc.dram_tensor("out_p", [B, L, DM], F32, kind="ExternalOutput")

    with tile.TileContext(nc) as tc:
        with (
            tc.tile_pool(name="weights", bufs=1) as wp,
            tc.tile_pool(name="resident", bufs=1) as rp,
            tc.tile_pool(name="dram", bufs=1, space="DRAM") as dp,
            tc.tile_pool(name="pa", bufs=2) as pa,
            tc.tile_pool(name="pb", bufs=2) as pb,
            tc.tile_pool(name="ps_in", bufs=2, space="PSUM") as ps_in,
            tc.tile_pool(name="ps_cv", bufs=1, space="PSUM") as ps_cv,
            tc.tile_pool(name="ps_small", bufs=1, space="PSUM") as ps_small,
            tc.tile_pool(name="ps_y", bufs=1, space="PSUM") as ps_y,
        ):
            # ---- weights to SBUF ----
            win_sb = wp.tile([128, KBLK, 2 * DIL], BF16)
            nc.sync.dma_start(win_sb[:], win_d[:].rearrange("(k p) m -> p k m", p=128))
            wout_sb = wp.tile([128, NBLK, DM], BF16)
            nc.sync.dma_start(wout_sb[:], wout_d[:].rearrange("(k p) m -> p k m", p=128))
            wx_sb = wp.tile([128, NBLK, 2 * DS], BF16)
            nc.sync.dma_start(wx_sb[:], wx_d[:].rearrange("(k p) m -> p k m", p=128))
            wdt_sb = wp.tile([DS, DIL], BF16)
            nc.sync.dma_start(wdt_sb[:], wdt_d[:])
            a_sb = wp.tile([128, NBLK, DS], F32)
            nc.sync.dma_start(a_sb[:], a_d[:].rearrange("(k p) m -> p k m", p=128))
            convdiag_sb = wp.tile([128, NBLK, DC * 128], BF16)
            nc.sync.dma_start(convdiag_sb[:],
                              convdiag_d[:].rearrange("(k p) m -> p k m", p=128))
            convb_sb = wp.tile([128, NBLK, 1], F32)
            nc.sync.dma_start(convb_sb[:], convb_d[:].rearrange("(k p) m -> p k m", p=128))
            bdt_sb = wp.tile([128, NBLK, 1], F32)
            nc.sync.dma_start(bdt_sb[:], bdt_d[:].rearrange("(k p) m -> p k m", p=128))
            identb_sb = wp.tile([128, 128], BF16)
            nc.sync.dma_start(identb_sb[:], identb_d[:])
            diagd_sb = wp.tile([128, NBLK, 128], BF16)
            nc.sync.dma_start(diagd_sb[:], diagd_d[:].rearrange("(k p) m -> p k m", p=128))

            # ---- resident activations ----
            xcv = [[rp.tile([128, L], BF16, name=f"xcv{b_}{k}", tag=f"xcv{b_}{k}")
                    for k in range(NBLK)] for b_ in range(B)]
            zac = [[rp.tile([128, L], BF16, name=f"zac{b_}{k}", tag=f"zac{b_}{k}")
                    for k in range(NBLK)] for b_ in range(B)]
            # AllReduced x_ssm in DRAM: rows 0:DS = dt_in, rows DS: = B_ssm
            # (DRAM so the bb partition-broadcast DMA can read it)
            ccall = [dp.tile([2 * DS, L], BF16, name=f"ccall{b_}")
                     for b_ in range(B)]
            # dt_in rows staged in SBUF for the dt_proj matmul
            dtin_sb = [rp.tile([DS, L], BF16, name=f"dtin{b_}",
                               tag=f"dtin{b_}") for b_ in range(B)]
            md = [[rp.tile([128, L], BF16, name=f"md{b_}{k}", tag=f"md{b_}{k}")
                   for k in range(NBLK)] for b_ in range(B)]

            cc_in = [[dp.tile([2 * DS, LTA], BF16, name=f"cc_in{b_}{ch}")
                      for ch in range(NCHA)] for b_ in range(B)]
            cc_out = [[dp.tile([2 * DS, LTA], BF16, addr_space="Shared",
                               name=f"cc_out{b_}{ch}") for ch in range(NCHA)]
                      for b_ in range(B)]

            xp_buf = [pa.tile([128, LTA + DC - 1], BF16, name=f"xpb{k}",
                              tag=f"xpb{k}", bufs=1) for k in range(NBLK)]

            def emit_a_compute(b_, ch):
                """in_proj + conv + silu + x_proj partial + cc_in DMA for one
                512-token chunk of batch b_. (No collective here.)"""
                t0 = ch * LTA
                xs_all = pa.tile([128, KBLK, LTA], BF16, tag="xs_all", bufs=3)
                nc.sync.dma_start(
                    xs_all[:], x_t[b_].transpose([1, 0, 2])[:, :, t0:t0 + LTA])
                for m in range(2 * NBLK):
                    ps = ps_in.tile([128, LTA], F32, tag="ps_in")
                    for kb in range(KBLK):
                        nc.tensor.matmul(
                            ps[:], win_sb[:, kb, m * 128:(m + 1) * 128],
                            xs_all[:, kb, :],
                            start=(kb == 0), stop=(kb == KBLK - 1))
                    if m < NBLK:  # x branch: conv + silu
                        blk = m
                        if ch == 0:
                            nc.vector.memset(xp_buf[blk][:, 0:DC - 1], 0.0)
                        else:
                            nc.vector.tensor_copy(
                                xp_buf[blk][:, 0:DC - 1],
                                xp_buf[blk][:, LTA:LTA + DC - 1])
                        nc.scalar.copy(xp_buf[blk][:, DC - 1:LTA + DC - 1], ps[:])
                        psc = ps_cv.tile([128, LTA], F32, tag="ps_cv")
                        for k in range(DC):
                            nc.tensor.matmul(
                                psc[:],
                                convdiag_sb[:, blk, k * 128:(k + 1) * 128],
                                xp_buf[blk][:, k:k + LTA],
                                start=(k == 0), stop=(k == DC - 1))
                        nc.scalar.activation(
                            xcv[b_][blk][:, t0:t0 + LTA], psc[:],
                            AF.Silu, bias=convb_sb[:, blk, :])
                        if m == NBLK - 1:
                            # x_proj partial for this chunk
                            psx = ps_small.tile([128, LTA], F32, tag="ps_small")
                            for kb in range(NBLK):
                                nc.tensor.matmul(
                                    psx[0:2 * DS, :], wx_sb[:, kb, :],
                                    xcv[b_][kb][:, t0:t0 + LTA],
                                    start=(kb == 0), stop=(kb == NBLK - 1))
                            xssb = pa.tile([2 * DS, LTA], BF16, tag="xssb",
                                           bufs=2)
                            nc.scalar.copy(xssb[:], psx[0:2 * DS, :])
                            nc.sync.dma_start(cc_in[b_][ch][:], xssb[:])
                    else:  # z branch: silu
                        blk = m - NBLK
                        nc.scalar.activation(
                            zac[b_][blk][:, t0:t0 + LTA], ps[:], AF.Silu)

            def emit_a_reduce(b_, ch):
                """AllReduce for one chunk, repack into ccall, then dt:
                md = softplus(dt_raw + b_dt); A is sign-negated on the host
                so da = exp(A * md) is the decay directly."""
                t0 = ch * LTA
                nc.gpsimd.collective_compute(
                    "AllReduce", OP.add,
                    ins=[cc_in[b_][ch].opt()],
                    outs=[cc_out[b_][ch].opt()],
                    replica_groups=[list(range(NCORES))])
                nc.sync.dma_start(ccall[b_][:, t0:t0 + LTA], cc_out[b_][ch][:])
                nc.sync.dma_start(dtin_sb[b_][:, t0:t0 + LTA],
                                  cc_out[b_][ch][0:DS, :])
                for blk2 in range(NBLK):
                    psd = ps_small.tile([128, LTA], F32, tag="ps_small")
                    nc.tensor.matmul(
                        psd[:], wdt_sb[:, blk2 * 128:(blk2 + 1) * 128],
                        dtin_sb[b_][:, t0:t0 + LTA],
                        start=True, stop=True)
                    # md = ln(sigmoid(-(dt_raw + b_dt))) = -softplus(.)
                    nc.scalar.activation(
                        md[b_][blk2][:, t0:t0 + LTA], psd[:],
                        AF.Sigmoid, bias=bdt_sb[:, blk2, :], scale=-1.0)
                    nc.scalar.activation(
                        md[b_][blk2][:, t0:t0 + LTA],
                        md[b_][blk2][:, t0:t0 + LTA], AF.Ln)

            HL = L // 2

            def emit_dtx(b_):
                """dtx = md * xcv, emitted per L-half so the low half is
                available as soon as md's first two chunks land."""
                dtx = [pb.tile([128, L], BF16, tag=f"dtx{blk}", bufs=1,
                               name=f"dtx{b_}{blk}") for blk in range(NBLK)]
                for half in range(2):
                    s = slice(half * HL, (half + 1) * HL)
                    for blk in range(NBLK):
                        nc.vector.tensor_mul(dtx[blk][:, s], md[b_][blk][:, s],
                                             xcv[b_][blk][:, s])
                return dtx

            def emit_phase_b_n(b_, blk, n, dtx, y_ps):
                """SSM channel n for one channel-block of batch b_,
                processed in chained L-halves for earlier pipeline start."""
                bb = pb.tile([128, L], BF16, tag="bbn", bufs=3,
                             name=f"bb{b_}{blk}{n}")
                da = pb.tile([128, L], F32, tag="dan", bufs=3,
                             name=f"da{b_}{blk}{n}")
                u = pb.tile([128, L], BF16, tag="un", bufs=3,
                            name=f"u{b_}{blk}{n}")
                h = pb.tile([128, L], BF16, tag="hn", bufs=3,
                            name=f"h{b_}{blk}{n}")
                for half in range(2):
                    s = slice(half * HL, (half + 1) * HL)
                    nc.sync.dma_start(
                        bb[:, s],
                        ccall[b_][DS + n:DS + n + 1, s].broadcast_to([128, HL]))
                    # dA_n = exp(A[:, n] * md)   (md = -dt)
                    nc.scalar.activation(da[:, s], md[b_][blk][:, s], AF.Exp,
                                         scale=a_sb[:, blk, n:n + 1])
                    # u_n = dtx * B_n (DVE: gpsimd shares DVE's 2nd SBUF
                    # port, so gpsimd tensor ops would block the scans)
                    nc.vector.tensor_mul(u[:, s], dtx[blk][:, s], bb[:, s])
                    # chained scan halves
                    nc.vector.tensor_tensor_scan(
                        h[:, s], da[:, s], u[:, s],
                        0.0 if half == 0 else h[:, HL - 1:HL],
                        OP.mult, OP.add)
                    # y += h_n on the tensor engine (identity matmul)
                    for pt in (0, 1):
                        gpt = half * 2 + pt
                        nc.tensor.matmul(
                            y_ps[gpt][:], identb_sb[:],
                            h[:, gpt * 512:(gpt + 1) * 512],
                            start=(n == 0), stop=False)

            def emit_gate(b_, blk, y_ps):
                yin = pb.tile([128, L], BF16, tag=f"yin{blk}", bufs=1,
                              name=f"yin{b_}{blk}")
                for pt in range(NPT):
                    # y += x_conv * D via diag(D) matmul, then gate
                    nc.tensor.matmul(
                        y_ps[pt][:], diagd_sb[:, blk, :],
                        xcv[b_][blk][:, pt * 512:(pt + 1) * 512],
                        start=False, stop=True)
                    nc.vector.tensor_mul(
                        yin[:, pt * 512:(pt + 1) * 512], y_ps[pt][:],
                        zac[b_][blk][:, pt * 512:(pt + 1) * 512])
                return yin

            def emit_out_proj(b_, yins):
                # pipelined through the (now free) y PSUM banks, 2 per mt
                for mt in range(L // 128):
                    pso = [ps_y.tile([128, 512], F32, tag=f"y{(2 * mt + i) % 4}",
                                     name=f"pso{b_}{mt}{i}") for i in range(2)]
                    for dmh in range(2):
                        for blk in range(NBLK):
                            nc.tensor.matmul(
                                pso[dmh][:],
                                yins[blk][:, mt * 128:(mt + 1) * 128],
                                wout_sb[:, blk, dmh * 512:(dmh + 1) * 512],
                                start=(blk == 0), stop=(blk == NBLK - 1))
                    osb = pb.tile([128, DM], F32, tag="osb")
                    nc.scalar.copy(osb[:, 0:512], pso[0][:])
                    nc.scalar.copy(osb[:, 512:DM], pso[1][:])
                    nc.sync.dma_start(
                        out_d[b_, mt * 128:(mt + 1) * 128, :], osb[:])

            # ================= schedule =================
            for ch in range(NCHA):
                emit_a_compute(0, ch)
                emit_a_reduce(0, ch)

            # interleave: phase B(b0) with phase A(b1) compute chunks
            dtx0 = emit_dtx(0)
            yins0 = {}
            for blk in range(NBLK):
                y_ps = [ps_y.tile([128, 512], F32, tag=f"y{pt}", bufs=1,
                                  name=f"yps0{blk}{pt}") for pt in range(NPT)]
                for n in range(16):
                    emit_phase_b_n(0, blk, n, dtx0, y_ps)
                    if blk == 0 and n in (1, 5, 9, 13):
                        emit_a_compute(1, (n - 1) // 4)
                    if blk == 0 and n in (3, 7, 11, 15):
                        emit_a_reduce(1, (n - 3) // 4)
                yins0[blk] = emit_gate(0, blk, y_ps)
                if blk == 0:
                    dtx1 = emit_dtx(1)
            emit_out_proj(0, yins0)

            yins1 = {}
            for blk in range(NBLK):
                y_ps = [ps_y.tile([128, 512], F32, tag=f"y{pt}", bufs=1,
                                  name=f"yps1{blk}{pt}") for pt in range(NPT)]
                for n in range(16):
                    emit_phase_b_n(1, blk, n, dtx1, y_ps)
                yins1[blk] = emit_gate(1, blk, y_ps)
            emit_out_proj(1, yins1)

    nc.compile()
    return nc


_NC_CACHE = {}


def _get_nc():
    if "nc" not in _NC_CACHE:
        _NC_CACHE["nc"] = build_nc()
    return _NC_CACHE["nc"]


def make_in_maps(x, W_in, conv_w, conv_b, W_x, W_dt, b_dt, A_log, D, W_out):
    x = np.asarray(x, np.float32)
    W_in = np.asarray(W_in, np.float32)
    conv_w = np.asarray(conv_w, np.float32)
    conv_b = np.asarray(conv_b, np.float32)
    W_x = np.asarray(W_x, np.float32)
    W_dt = np.asarray(W_dt, np.float32)
    b_dt = np.asarray(b_dt, np.float32)
    A_log = np.asarray(A_log, np.float32)
    D = np.asarray(D, np.float32)
    W_out = np.asarray(W_out, np.float32)

    xt = np.ascontiguousarray(x.transpose(0, 2, 1)).reshape(B, KBLK, 128, L).astype(ml_dtypes.bfloat16)
    A = np.exp(A_log)  # positive |A|; md = -softplus(dt) on device

    in_maps = []
    for c in range(NCORES):
        lo = c * DIL
        sl = slice(lo, lo + DIL)
        # diag conv weights: [NBLK, DC, 128, 128] -> [DIL, DC*128]
        cd = np.zeros((NBLK, DC, 128, 128), np.float32)
        for blk in range(NBLK):
            for k in range(DC):
                np.fill_diagonal(cd[blk, k], conv_w[lo + blk * 128:
                                                    lo + (blk + 1) * 128, k])
        cd = cd.transpose(0, 2, 1, 3).reshape(DIL, DC * 128)
        in_maps.append({
            "x_t": xt,
            "win": np.ascontiguousarray(
                np.concatenate([W_in[:, sl], W_in[:, DI + lo:DI + lo + DIL]],
                               axis=1)).astype(ml_dtypes.bfloat16),
            "wout": np.ascontiguousarray(W_out[sl]).astype(ml_dtypes.bfloat16),
            "wx": np.ascontiguousarray(
                np.concatenate([W_x[sl, :DS], -W_x[sl, DS:]], axis=1)
            ).astype(ml_dtypes.bfloat16),
            "wdt": np.ascontiguousarray(W_dt[:, sl]).astype(ml_dtypes.bfloat16),
            "a": np.ascontiguousarray(A[sl]),
            "convdiag": np.ascontiguousarray(cd).astype(ml_dtypes.bfloat16),
            "convb": np.ascontiguousarray(conv_b[sl, None]),
            "bdt": np.ascontiguousarray(-b_dt[sl, None]),
            "identb": np.eye(128, dtype=ml_dtypes.bfloat16),
            "diagd": np.stack([np.diag(D[lo + k * 128:lo + (k + 1) * 128])
                               for k in range(NBLK)]).reshape(DIL, 128)
                       .astype(ml_dtypes.bfloat16),
        })
    return in_maps


def kernel(**inputs):
    nc = _get_nc()
    in_maps = make_in_maps(**inputs)
    res = run_bass_kernel_spmd(nc, in_maps, list(range(NCORES)))
    out = np.zeros((B, L, DM), np.float32)
    for c in range(NCORES):
        out += res.results[c]["out_p"]
    return out


# revision 18
# speedup vs baseline: 1.5071x; 1.1098x over previous
"""Mamba block (MockMambaBlock) on 8 Trainium2 NeuronCores.

Sharding: tensor-parallel over d_inner (8 x 256 channels), both batches on
every core. The x_proj/dt_proj contraction over d_inner is completed with an
on-device AllReduce (chunked per 512 tokens so it overlaps phase A); out_proj
row-partials are summed on the host (the gather step).

v3 layout of work across engines:
  - PE: in_proj, depthwise conv (diag-matmul), x_proj, dt_proj, y n-sum
    (identity matmul), D-term (diag matmul), out_proj.
  - Scalar: silu/softplus/exp activations, PSUM->SBUF copies.
  - DVE (vector): the 64 SSM scans (f32), dtx, 1/4 of u-multiplies, gating.
  - GPSIMD: 3/4 of the u = dtx*B multiplies (bf16 tensor_tensor).
Batches are pipelined: phase B of batch 0 is emitted interleaved with
phase A of batch 1 (compute only; its AllReduces are emitted at the
blk0/blk1 boundary so they never head-of-line-block gpsimd u-muls).
dt uses AF.Softplus with sign-negated A so da = exp(A*dt) directly.
"""

import sys

sys.path.insert(0, "/opt/trn_rl_repo")

import numpy as np
import ml_dtypes

import concourse.bass as bass
import concourse.bacc as bacc
import concourse.mybir as mybir
import concourse.tile as tile
from concourse.bass_utils import run_bass_kernel_spmd

F32 = mybir.dt.float32
BF16 = mybir.dt.bfloat16
AF = mybir.ActivationFunctionType
OP = mybir.AluOpType

B, L, DM, DI, DS, DC = 2, 2048, 1024, 2048, 16, 4
NCORES = 8
DIL = DI // NCORES          # 256 channels per core
NBLK = DIL // 128           # 2 partition blocks of channels
KBLK = DM // 128            # 8 contraction blocks for in_proj
LTA = 512                   # phase A token chunk
NCHA = L // LTA
NPT = L // 512


def build_nc():
    nc = bacc.Bacc()

    x_t = nc.dram_tensor("x_t", [B, KBLK, 128, L], BF16, kind="ExternalInput")
    win_d = nc.dram_tensor("win", [DM, 2 * DIL], BF16, kind="ExternalInput")
    wout_d = nc.dram_tensor("wout", [DIL, DM], BF16, kind="ExternalInput")
    wx_d = nc.dram_tensor("wx", [DIL, 2 * DS], BF16, kind="ExternalInput")
    wdt_d = nc.dram_tensor("wdt", [DS, DIL], BF16, kind="ExternalInput")
    a_d = nc.dram_tensor("a", [DIL, DS], F32, kind="ExternalInput")
    convdiag_d = nc.dram_tensor("convdiag", [DIL, DC * 128], BF16,
                                kind="ExternalInput")
    convb_d = nc.dram_tensor("convb", [DIL, 1], F32, kind="ExternalInput")
    bdt_d = nc.dram_tensor("bdt", [DIL, 1], F32, kind="ExternalInput")
    identb_d = nc.dram_tensor("identb", [128, 128], BF16, kind="ExternalInput")
    diagd_d = nc.dram_tensor("diagd", [DIL, 128], BF16, kind="ExternalInput")
    out_d = nc.dram_tensor("out_p", [B, L, DM], F32, kind="ExternalOutput")

    with tile.TileContext(nc) as tc:
        with (
            tc.tile_pool(name="weights", bufs=1) as wp,
            tc.tile_pool(name="resident", bufs=1) as rp,
            tc.tile_pool(name="dram", bufs=1, space="DRAM") as dp,
            tc.tile_pool(name="pa", bufs=2) as pa,
            tc.tile_pool(name="pb", bufs=2) as pb,
            tc.tile_pool(name="ps_in", bufs=2, space="PSUM") as ps_in,
            tc.tile_pool(name="ps_cv", bufs=1, space="PSUM") as ps_cv,
            tc.tile_pool(name="ps_small", bufs=1, space="PSUM") as ps_small,
            tc.tile_pool(name="ps_y", bufs=1, space="PSUM") as ps_y,
        ):
            # ---- weights to SBUF ----
            win_sb = wp.tile([128, KBLK, 2 * DIL], BF16)
            nc.sync.dma_start(win_sb[:], win_d[:].rearrange("(k p) m -> p k m", p=128))
            wout_sb = wp.tile([128, NBLK, DM], BF16)
            nc.sync.dma_start(wout_sb[:], wout_d[:].rearrange("(k p) m -> p k m", p=128))
            wx_sb = wp.tile([128, NBLK, 2 * DS], BF16)
            nc.sync.dma_start(wx_sb[:], wx_d[:].rearrange("(k p) m -> p k m", p=128))
            wdt_sb = wp.tile([DS, DIL], BF16)
            nc.sync.dma_start(wdt_sb[:], wdt_d[:])
            a_sb = wp.tile([128, NBLK, DS], F32)
            nc.sync.dma_start(a_sb[:], a_d[:].rearrange("(k p) m -> p k m", p=128))
            convdiag_sb = wp.tile([128, NBLK, DC * 128], BF16)
            nc.sync.dma_start(convdiag_sb[:],
                              convdiag_d[:].rearrange("(k p) m -> p k m", p=128))
            convb_sb = wp.tile([128, NBLK, 1], F32)
            nc.sync.dma_start(convb_sb[:], convb_d[:].rearrange("(k p) m -> p k m", p=128))
            bdt_sb = wp.tile([128, NBLK, 1], F32)
            nc.sync.dma_start(bdt_sb[:], bdt_d[:].rearrange("(k p) m -> p k m", p=128))
            identb_sb = wp.tile([128, 128], BF16)
            nc.sync.dma_start(identb_sb[:], identb_d[:])
            diagd_sb = wp.tile([128, NBLK, 128], BF16)
            nc.sync.dma_start(diagd_sb[:], diagd_d[:].rearrange("(k p) m -> p k m", p=128))

            # ---- resident activations ----
            xcv = [[rp.tile([128, L], BF16, name=f"xcv{b_}{k}", tag=f"xcv{b_}{k}")
                    for k in range(NBLK)] for b_ in range(B)]
            zac = [[rp.tile([128, L], BF16, name=f"zac{b_}{k}", tag=f"zac{b_}{k}")
                    for k in range(NBLK)] for b_ in range(B)]
            # AllReduced x_ssm in DRAM: rows 0:DS = dt_in, rows DS: = B_ssm
            # (DRAM so the bb partition-broadcast DMA can read it)
            ccall = [dp.tile([2 * DS, L], BF16, name=f"ccall{b_}")
                     for b_ in range(B)]
            # dt_in rows staged in SBUF for the dt_proj matmul
            dtin_sb = [rp.tile([DS, L], BF16, name=f"dtin{b_}",
                               tag=f"dtin{b_}") for b_ in range(B)]
            md = [[rp.tile([128, L], BF16, name=f"md{b_}{k}", tag=f"md{b_}{k}")
                   for k in range(NBLK)] for b_ in range(B)]

            cc_in = [[dp.tile([2 * DS, LTA], BF16, name=f"cc_in{b_}{ch}")
                      for ch in range(NCHA)] for b_ in range(B)]
            cc_out = [[dp.tile([2 * DS, LTA], BF16, addr_space="Shared",
                               name=f"cc_out{b_}{ch}") for ch in range(NCHA)]
                      for b_ in range(B)]

            xp_buf = [pa.tile([128, LTA + DC - 1], BF16, name=f"xpb{k}",
                              tag=f"xpb{k}", bufs=1) for k in range(NBLK)]

            def emit_a_compute(b_, ch):
                """in_proj + conv + silu + x_proj partial + cc_in DMA for one
                512-token chunk of batch b_. (No collective here.)"""
                t0 = ch * LTA
                xs_all = pa.tile([128, KBLK, LTA], BF16, tag="xs_all", bufs=3)
                nc.sync.dma_start(
                    xs_all[:], x_t[b_].transpose([1, 0, 2])[:, :, t0:t0 + LTA])
                for m in range(2 * NBLK):
                    ps = ps_in.tile([128, LTA], F32, tag="ps_in")
                    for kb in range(KBLK):
                        nc.tensor.matmul(
                            ps[:], win_sb[:, kb, m * 128:(m + 1) * 128],
                            xs_all[:, kb, :],
                            start=(kb == 0), stop=(kb == KBLK - 1))
                    if m < NBLK:  # x branch: conv + silu
                        blk = m
                        if ch == 0:
                            nc.vector.memset(xp_buf[blk][:, 0:DC - 1], 0.0)
                        else:
                            nc.vector.tensor_copy(
                                xp_buf[blk][:, 0:DC - 1],
                                xp_buf[blk][:, LTA:LTA + DC - 1])
                        nc.scalar.copy(xp_buf[blk][:, DC - 1:LTA + DC - 1], ps[:])
                        psc = ps_cv.tile([128, LTA], F32, tag="ps_cv")
                        for k in range(DC):
                            nc.tensor.matmul(
                                psc[:],
                                convdiag_sb[:, blk, k * 128:(k + 1) * 128],
                                xp_buf[blk][:, k:k + LTA],
                                start=(k == 0), stop=(k == DC - 1))
                        nc.scalar.activation(
                            xcv[b_][blk][:, t0:t0 + LTA], psc[:],
                            AF.Silu, bias=convb_sb[:, blk, :])
                        if m == NBLK - 1:
                            # x_proj partial for this chunk
                            psx = ps_small.tile([128, LTA], F32, tag="ps_small")
                            for kb in range(NBLK):
                                nc.tensor.matmul(
                                    psx[0:2 * DS, :], wx_sb[:, kb, :],
                                    xcv[b_][kb][:, t0:t0 + LTA],
                                    start=(kb == 0), stop=(kb == NBLK - 1))
                            xssb = pa.tile([2 * DS, LTA], BF16, tag="xssb",
                                           bufs=2)
                            nc.scalar.copy(xssb[:], psx[0:2 * DS, :])
                            nc.sync.dma_start(cc_in[b_][ch][:], xssb[:])
                    else:  # z branch: silu
                        blk = m - NBLK
                        nc.scalar.activation(
                            zac[b_][blk][:, t0:t0 + LTA], ps[:], AF.Silu)

            def emit_a_comm(b_, ch):
                """AllReduce for one chunk + repack into ccall/dtin."""
                t0 = ch * LTA
                nc.gpsimd.collective_compute(
                    "AllReduce", OP.add,
                    ins=[cc_in[b_][ch].opt()],
                    outs=[cc_out[b_][ch].opt()],
                    replica_groups=[list(range(NCORES))])
                nc.sync.dma_start(ccall[b_][:, t0:t0 + LTA], cc_out[b_][ch][:])
                nc.sync.dma_start(dtin_sb[b_][:, t0:t0 + LTA],
                                  cc_out[b_][ch][0:DS, :])

            def emit_a_dt(b_, ch):
                """dt for one chunk (emitted a chunk late so the Sigmoid
                never head-of-line-blocks the scalar queue on AR latency)."""
                t0 = ch * LTA
                for blk2 in range(NBLK):
                    psd = ps_small.tile([128, LTA], F32, tag="ps_small")
                    nc.tensor.matmul(
                        psd[:], wdt_sb[:, blk2 * 128:(blk2 + 1) * 128],
                        dtin_sb[b_][:, t0:t0 + LTA],
                        start=True, stop=True)
                    # md = ln(sigmoid(-(dt_raw + b_dt))) = -softplus(.)
                    nc.scalar.activation(
                        md[b_][blk2][:, t0:t0 + LTA], psd[:],
                        AF.Sigmoid, bias=bdt_sb[:, blk2, :], scale=-1.0)
                    nc.scalar.activation(
                        md[b_][blk2][:, t0:t0 + LTA],
                        md[b_][blk2][:, t0:t0 + LTA], AF.Ln)

            HL = L // 2

            def emit_dtx_half(b_, dtx, half):
                s = slice(half * HL, (half + 1) * HL)
                for blk in range(NBLK):
                    nc.vector.tensor_mul(dtx[blk][:, s], md[b_][blk][:, s],
                                         xcv[b_][blk][:, s])

            def alloc_dtx(b_):
                return [pb.tile([128, L], BF16, tag=f"dtx{blk}", bufs=1,
                                name=f"dtx{b_}{blk}") for blk in range(NBLK)]

            def emit_phase_b_half(b_, blk, n, half, dtx, y_ps, carry):
                """One L-half of SSM channel n for one channel-block.
                The low sweep (half=0) runs all n first and parks each
                chain's carry column; the high sweep resumes from it."""
                s = slice(half * HL, (half + 1) * HL)
                bb = pb.tile([128, HL], BF16, tag="bbn", bufs=3,
                             name=f"bb{b_}{blk}{n}{half}")
                da = pb.tile([128, HL], F32, tag="dan", bufs=4,
                             name=f"da{b_}{blk}{n}{half}")
                u = pb.tile([128, HL], BF16, tag="un", bufs=3,
                            name=f"u{b_}{blk}{n}{half}")
                h = pb.tile([128, HL], BF16, tag="hn", bufs=3,
                            name=f"h{b_}{blk}{n}{half}")
                nc.sync.dma_start(
                    bb[:],
                    ccall[b_][DS + n:DS + n + 1, s].broadcast_to([128, HL]))
                # dA_n = exp(A[:, n] * md)   (md = -dt)
                nc.scalar.activation(da[:], md[b_][blk][:, s], AF.Exp,
                                     scale=a_sb[:, blk, n:n + 1])
                # u_n = dtx * B_n (DVE only: gpsimd shares DVE's 2nd SBUF
                # port, so gpsimd tensor ops would block the scans)
                nc.vector.tensor_mul(u[:], dtx[blk][:, s], bb[:])
                nc.vector.tensor_tensor_scan(
                    h[:], da[:], u[:],
                    0.0 if half == 0 else carry[:, n:n + 1],
                    OP.mult, OP.add)
                if half == 0:
                    # park the chain state for the high sweep
                    nc.vector.tensor_copy(carry[:, n:n + 1], h[:, HL - 1:HL])
                # y += h_n on the tensor engine (identity matmul)
                for pt in (0, 1):
                    gpt = half * 2 + pt
                    nc.tensor.matmul(
                        y_ps[gpt][:], identb_sb[:],
                        h[:, pt * 512:(pt + 1) * 512],
                        start=(n == 0), stop=False)

            def emit_gate_pt(b_, blk, pt, y_ps, yin):
                # y += x_conv * D via diag(D) matmul, then gate
                nc.tensor.matmul(
                    y_ps[pt][:], diagd_sb[:, blk, :],
                    xcv[b_][blk][:, pt * 512:(pt + 1) * 512],
                    start=False, stop=True)
                nc.vector.tensor_mul(
                    yin[:, pt * 512:(pt + 1) * 512], y_ps[pt][:],
                    zac[b_][blk][:, pt * 512:(pt + 1) * 512])

            def emit_out_proj_mt(b_, yins, mt):
                # out_proj pipelined through the idle ps_in banks
                pso = [ps_in.tile([128, 512], F32, tag="ps_in",
                                  name=f"pso{b_}{mt}{i}") for i in range(2)]
                for dmh in range(2):
                    for blk in range(NBLK):
                        nc.tensor.matmul(
                            pso[dmh][:],
                            yins[blk][:, mt * 128:(mt + 1) * 128],
                            wout_sb[:, blk, dmh * 512:(dmh + 1) * 512],
                            start=(blk == 0), stop=(blk == NBLK - 1))
                osb = pb.tile([128, DM], F32, tag="osb")
                nc.scalar.copy(osb[:, 0:512], pso[0][:])
                nc.scalar.copy(osb[:, 512:DM], pso[1][:])
                nc.sync.dma_start(
                    out_d[b_, mt * 128:(mt + 1) * 128, :], osb[:])

            def emit_phase_b_blk(b_, blk, dtx, interleave=None):
                """Full phase B for one channel-block: low sweep over all n,
                then high sweep. `interleave` maps (half, n) -> callback."""
                y_ps = [ps_y.tile([128, 512], F32, tag=f"y{pt}", bufs=1,
                                  name=f"yps{b_}{blk}{pt}") for pt in range(NPT)]
                carry = pb.tile([128, 16], F32, tag="carry", bufs=2,
                                name=f"carry{b_}{blk}")
                for half in range(2):
                    for n in range(16):
                        emit_phase_b_half(b_, blk, n, half, dtx, y_ps, carry)
                        if interleave and (half, n) in interleave:
                            interleave[(half, n)]()
                yin = pb.tile([128, L], BF16, tag=f"yin{blk}", bufs=1,
                              name=f"yin{b_}{blk}")
                for pt in range(NPT):
                    emit_gate_pt(b_, blk, pt, y_ps, yin)
                return yin

            # ================= schedule =================
            # A(b0): dt is emitted one chunk late so Sigmoid never waits
            # at the scalar queue head on AR latency.
            for ch in range(NCHA):
                emit_a_compute(0, ch)
                emit_a_comm(0, ch)
                if ch > 0:
                    emit_a_dt(0, ch - 1)
            emit_a_dt(0, NCHA - 1)

            dtx0 = alloc_dtx(0)
            dtx1 = alloc_dtx(1)
            emit_dtx_half(0, dtx0, 0)
            emit_dtx_half(0, dtx0, 1)

            # b0/blk0 with A(b1) interleaved: computes spread over the low
            # sweep, comms right after, dt one position later.
            il = {
                (0, 2): lambda: emit_a_compute(1, 0),
                (0, 5): lambda: (emit_a_compute(1, 1), emit_a_comm(1, 0)),
                (0, 8): lambda: (emit_a_compute(1, 2), emit_a_comm(1, 1),
                                 emit_a_dt(1, 0)),
                (0, 11): lambda: (emit_a_compute(1, 3), emit_a_comm(1, 2),
                                  emit_a_dt(1, 1)),
                (0, 14): lambda: (emit_a_comm(1, 3), emit_a_dt(1, 2)),
                (1, 1): lambda: emit_a_dt(1, 3),
                (1, 4): lambda: emit_dtx_half(1, dtx1, 0),
                (1, 8): lambda: emit_dtx_half(1, dtx1, 1),
            }
            yins0 = {}
            yins1 = {}
            yins0[0] = emit_phase_b_blk(0, 0, dtx0, il)
            yins0[1] = emit_phase_b_blk(0, 1, dtx0)
            for mt in range(L // 128):
                emit_out_proj_mt(0, yins0, mt)
            yins1[0] = emit_phase_b_blk(1, 0, dtx1)
            yins1[1] = emit_phase_b_blk(1, 1, dtx1)
            for mt in range(L // 128):
                emit_out_proj_mt(1, yins1, mt)

    nc.compile()
    return nc


_NC_CACHE = {}


def _get_nc():
    if "nc" not in _NC_CACHE:
        _NC_CACHE["nc"] = build_nc()
    return _NC_CACHE["nc"]


def make_in_maps(x, W_in, conv_w, conv_b, W_x, W_dt, b_dt, A_log, D, W_out):
    x = np.asarray(x, np.float32)
    W_in = np.asarray(W_in, np.float32)
    conv_w = np.asarray(conv_w, np.float32)
    conv_b = np.asarray(conv_b, np.float32)
    W_x = np.asarray(W_x, np.float32)
    W_dt = np.asarray(W_dt, np.float32)
    b_dt = np.asarray(b_dt, np.float32)
    A_log = np.asarray(A_log, np.float32)
    D = np.asarray(D, np.float32)
    W_out = np.asarray(W_out, np.float32)

    xt = np.ascontiguousarray(x.transpose(0, 2, 1)).reshape(B, KBLK, 128, L).astype(ml_dtypes.bfloat16)
    A = np.exp(A_log)  # positive |A|; md = -softplus(dt) on device

    in_maps = []
    for c in range(NCORES):
        lo = c * DIL
        sl = slice(lo, lo + DIL)
        # diag conv weights: [NBLK, DC, 128, 128] -> [DIL, DC*128]
        cd = np.zeros((NBLK, DC, 128, 128), np.float32)
        for blk in range(NBLK):
            for k in range(DC):
                np.fill_diagonal(cd[blk, k], conv_w[lo + blk * 128:
                                                    lo + (blk + 1) * 128, k])
        cd = cd.transpose(0, 2, 1, 3).reshape(DIL, DC * 128)
        in_maps.append({
            "x_t": xt,
            "win": np.ascontiguousarray(
                np.concatenate([W_in[:, sl], W_in[:, DI + lo:DI + lo + DIL]],
                               axis=1)).astype(ml_dtypes.bfloat16),
            "wout": np.ascontiguousarray(W_out[sl]).astype(ml_dtypes.bfloat16),
            "wx": np.ascontiguousarray(
                np.concatenate([W_x[sl, :DS], -W_x[sl, DS:]], axis=1)
            ).astype(ml_dtypes.bfloat16),
            "wdt": np.ascontiguousarray(W_dt[:, sl]).astype(ml_dtypes.bfloat16),
            "a": np.ascontiguousarray(A[sl]),
            "convdiag": np.ascontiguousarray(cd).astype(ml_dtypes.bfloat16),
            "convb": np.ascontiguousarray(conv_b[sl, None]),
            "bdt": np.ascontiguousarray(-b_dt[sl, None]),
            "identb": np.eye(128, dtype=ml_dtypes.bfloat16),
            "diagd": np.stack([np.diag(D[lo + k * 128:lo + (k + 1) * 128])
                               for k in range(NBLK)]).reshape(DIL, 128)
                       .astype(ml_dtypes.bfloat16),
        })
    return in_maps


def kernel(**inputs):
    nc = _get_nc()
    in_maps = make_in_maps(**inputs)
    res = run_bass_kernel_spmd(nc, in_maps, list(range(NCORES)))
    out = np.zeros((B, L, DM), np.float32)
    for c in range(NCORES):
        out += res.results[c]["out_p"]
    return out
